# revision 24
# baseline (speedup 1.0000x reference)
"""TRN2 Bass kernel for nn_AlgebraicBlock (dense transformer block):
MR-norm -> QKV -> ALiBi attention w/ rational softmax -> out-proj residual ->
MR-norm -> rational SwiGLU FFN -> residual.   x: [1, 2048, 1024] f32.

Tensor-parallel over 8 NeuronCores, software-pipelined at 512-column
(t-chunk) granularity, 2 collectives per chunk (v1 had 3):

  it:   qkv+scores(it) -> [oproj+FFN1(it-1) interleaved with AV(it)] ->
        AllGather(attn it) -> FFN2(it-1) -> ReduceScatter(it-1) ->
        residual(it-2) -> DMA out

Key structural points vs v1:
- The FULL x2 = x + attn@w_out.T is computed redundantly on every core
  (kills v1's x2 AllGather); the x-residual rides the out-proj PSUM
  accumulation as an identity-matmul term.
- Causal mask applied as a -10000 bias identity-matmul into the score
  PSUM (like the reference), not a 0/1 multiply pass.
- Attention AV matmuls are emitted in two waves AFTER oproj / FFN1 so the
  FIFO tensor queue never waits on the vector softmax chain.
- The warmup collective is the very first instruction group so the
  one-time CC-init barrier overlaps weight DMAs and chunk-0 compute.

Rational softmax: RECIPROCAL_APPROX_FAST(1+|s|) then ANT_P4(s, r) =
((s*r+1)^2)^2 = 16*rsig(s)^4.  FFN rsig uses ANT_GSR(g, r) = (g*r+1)*g on
the unnormalized gate with b2 = mean|x2|+eps + |g|; inv2^2/2 rides the val
path (0.5 folded into w3 host-side).  All GEMMs bf16, f32 PSUM accum,
transposed [feature, T] layout.
"""

import numpy as np
import ml_dtypes

T, C, H, D, F = 2048, 1024, 16, 64, 4096
NCORES = 8
EPS = 1e-6
P = 128
TT = T // 512          # 4 t-chunks of 512
CS = C // P            # 8 c-subtiles
BF = ml_dtypes.bfloat16

TRACE = False          # set True by test.py for neuron-profile timing
LAST_RESULTS = None    # BassKernelResults of the last run (for test.py)

_PROGRAM = None
_DVE_OPS = None


def _bf16(x):
    return np.asarray(x, dtype=BF)


def _alibi_slopes():
    start = 2.0 ** (-8.0 / H)
    return (start ** np.arange(1, H + 1)).astype(np.float64)


def _register_dve_ops():
    """Register the two fused softmax/ffn DVE ops (idempotent)."""
    global _DVE_OPS
    if _DVE_OPS is not None:
        return _DVE_OPS
    import concourse.dve_ops as dops
    from concourse.dve_spec import Spec, Src0, Src1, One, lower, sq
    from concourse.dve_uop import DveOpSpec

    def _p4ref(in0, in1, c0, c1, c2):
        t = in0.astype(np.float32) * in1.astype(np.float32)
        return (((t + 1.0) ** 2) ** 2).astype(np.float32)

    def _gsrref(in0, in1, c0, c1, c2):
        t = in0.astype(np.float32) * in1.astype(np.float32)
        return ((t + 1.0) * in0.astype(np.float32)).astype(np.float32)

    defs = [
        ("ANT_P4", Spec(body=sq(sq(Src0 * Src1 + One)), reference=_p4ref)),
        ("ANT_GSR", Spec(body=(Src0 * Src1 + One) * Src0, reference=_gsrref)),
    ]
    made = []
    for name, spec in defs:
        if name in dops._SUB_OPCODE_FOR_NAME:
            made.append(next(o for o in dops.OPS if o.name == name))
            continue
        row = max(dops._SUB_OPCODE_FOR_NAME.values()) + 1
        assert row < 0x20
        dops._SUB_OPCODE_FOR_NAME[name] = row
        shas = {}
        for ver in ("v3", "v4"):
            u = lower(spec, ver=ver)
            shas[ver] = DveOpSpec(name=name, opcode=row, uops=u,
                                  rd1_en=True).sha(ver)
        op = dops.DveOp(name, spec, subdim=False, uops_sha=shas)
        dops.OPS.append(op)
        dops.CUSTOM_DVE_SPECS[name] = spec
        made.append(op)
    _DVE_OPS = tuple(made)
    return _DVE_OPS


def _prepare_in_maps(x, w_qkv, w_out, w_merged, w3, norm1_w, norm2_w):
    """Host-side sharding + weight preprocessing (layout/precision only)."""
    x = np.asarray(x, np.float32)[0]            # [T, C]
    xT = np.ascontiguousarray(x.T)              # [C, T]
    xt_bf = _bf16(xT)
    slopes = _alibi_slopes()

    pos = np.arange(T, dtype=np.float64)
    t_hi, t_lo = pos // 64, pos % 64

    Wn = np.asarray(w_qkv, np.float32) * np.asarray(norm1_w, np.float32)[None, :]
    w_out = np.asarray(w_out, np.float32)
    wm_n = np.asarray(w_merged, np.float32) * np.asarray(norm2_w, np.float32)[None, :]
    w3 = np.asarray(w3, np.float32) * 0.5       # rsig half-factor folded here

    in_maps = []
    for i in range(NCORES):
        h0, h1 = 2 * i, 2 * i + 1
        rows = []
        for h in (h0, h1):
            rows.append(Wn[64 * h:64 * h + 64, :] * 0.125)          # q (scaled)
        for h in (h0, h1):
            rows.append(Wn[C + 64 * h:C + 64 * h + 64, :])          # k
        for h in (h0, h1):
            rows.append(Wn[2 * C + 64 * h:2 * C + 64 * h + 64, :])  # v
        wqkv_t = _bf16(np.ascontiguousarray(np.concatenate(rows, 0).T))  # [1024, 384]

        wout_t = _bf16(np.ascontiguousarray(w_out[P * i:P * (i + 1), :].T))  # [1024, 128]

        gsl = slice(512 * i, 512 * (i + 1))
        wm = np.concatenate([wm_n[gsl, :], wm_n[F:][gsl, :]], 0)     # [1024, C]
        wm_t = _bf16(np.ascontiguousarray(wm.T))                     # [1024, 1024]
        w3_t = _bf16(np.ascontiguousarray(w3[:, gsl].T))             # [512, 1024]

        aug_q = np.zeros((2, 4, T), np.float64)
        aug_k = np.zeros((2, 4, T), np.float64)
        for j, h in enumerate((h0, h1)):
            sl = float(_bf16(slopes[h]))
            aug_q[j, 0] = -t_hi
            aug_q[j, 1] = -t_lo
            aug_q[j, 2] = sl * 64
            aug_q[j, 3] = sl
            aug_k[j, 0] = sl * 64
            aug_k[j, 1] = sl
            aug_k[j, 2] = t_hi
            aug_k[j, 3] = t_lo

        # causal-mask BIAS tiles: -10000 where key > query, else 0.
        # maskb[rr][ss, tq]: key = 128*(4c+rr)+ss vs query = 512c+tq
        # -> masked iff 128*rr + ss > tq (c cancels).
        maskb = np.zeros((4, P, 512), np.float64)
        for rr in range(4):
            s_idx = P * rr + np.arange(P)[:, None]
            maskb[rr] = np.where(s_idx > np.arange(512)[None, :], -10000.0, 0.0)

        in_maps.append({
            "xt_bf": xt_bf,
            "xt_sh": np.ascontiguousarray(xT[P * i:P * (i + 1)]),
            "wqkv_t": wqkv_t,
            "wout_t": wout_t,
            "wm_t": wm_t,
            "w3_t": w3_t,
            "aug_q": _bf16(aug_q),
            "aug_k": _bf16(aug_k),
            "maskb": _bf16(maskb),
        })
    return in_maps


def _build_program():
    import concourse.bass as bass
    import concourse.mybir as mybir
    import concourse.tile as tile
    from concourse import bacc
    from concourse.masks import make_identity

    P4_OP, GSR_OP = _register_dve_ops()

    dt = mybir.dt
    Alu = mybir.AluOpType
    Act = mybir.ActivationFunctionType

    nc = bacc.Bacc("TRN2", target_bir_lowering=False, debug=False,
                   enable_asserts=True, num_devices=NCORES)

    # I/O
    xt_bf = nc.dram_tensor("xt_bf", [C, T], dt.bfloat16, kind="ExternalInput")
    xt_sh = nc.dram_tensor("xt_sh", [P, T], dt.float32, kind="ExternalInput")
    wqkv_t = nc.dram_tensor("wqkv_t", [C, 384], dt.bfloat16, kind="ExternalInput")
    wout_t = nc.dram_tensor("wout_t", [C, P], dt.bfloat16, kind="ExternalInput")
    wm_t = nc.dram_tensor("wm_t", [C, 1024], dt.bfloat16, kind="ExternalInput")
    w3_t = nc.dram_tensor("w3_t", [512, 1024], dt.bfloat16, kind="ExternalInput")
    aug_q = nc.dram_tensor("aug_q", [2, 4, T], dt.bfloat16, kind="ExternalInput")
    aug_k = nc.dram_tensor("aug_k", [2, 4, T], dt.bfloat16, kind="ExternalInput")
    maskb = nc.dram_tensor("maskb", [4, P, 512], dt.bfloat16, kind="ExternalInput")
    out = nc.dram_tensor("out", [P, T], dt.float32, kind="ExternalOutput")

    # internal DRAM (collective bounces)
    warm_in = nc.dram_tensor("warm_in", [1, 64], dt.float32)
    warm_out = nc.dram_tensor("warm_out", [1, 512], dt.float32, addr_space="Shared")
    attn_in = nc.dram_tensor("attn_in", [TT, P, 512], dt.bfloat16)
    attn_out = nc.dram_tensor("attn_out", [TT, C, 512], dt.bfloat16, addr_space="Shared")
    h2_in = nc.dram_tensor("h2_in", [TT, P, 512], dt.bfloat16)
    h2_out = nc.dram_tensor("h2_out", [TT, C, 512], dt.bfloat16, addr_space="Shared")
    rs_in = nc.dram_tensor("rs_in", [TT, C, 512], dt.bfloat16)
    rs_out = nc.dram_tensor("rs_out", [TT, P, 512], dt.bfloat16)

    RG = [list(range(NCORES))]
    NSLOT = 3  # x / gathered-x2 chunk ring

    with tile.TileContext(nc, num_cores=NCORES) as tc:
        with (
            tc.tile_pool(name="sb", bufs=1) as sb,
            tc.tile_pool(name="wk", bufs=2) as wk,
            tc.tile_pool(name="psS", bufs=3, space="PSUM") as psS,
            tc.tile_pool(name="psV", bufs=2, space="PSUM") as psV,
            tc.tile_pool(name="psF", bufs=3, space="PSUM") as psF,
        ):
            # ---------- warmup collective FIRST ----------
            wtiny = sb.tile([1, 64], dt.float32, tag="wtiny")
            nc.vector.memset(wtiny[:], 0.0)
            nc.sync.dma_start(warm_in.ap(), wtiny[:])
            nc.gpsimd.collective_compute(
                "AllGather", Alu.bypass, replica_groups=RG,
                ins=[warm_in.ap()], outs=[warm_out.ap()])

            # ---------- constants + weight DMA ----------
            # DMA order matters for the first-iteration critical path:
            # qkv weights + x chunk 0 first.
            ident = sb.tile([P, P], dt.bfloat16, tag="ident")
            make_identity(nc, ident[:])
            ones128 = sb.tile([P, P], dt.bfloat16, tag="ones")
            nc.vector.memset(ones128[:], 1.0)
            ones164 = sb.tile([1, 64], dt.bfloat16, tag="ones164")
            nc.vector.memset(ones164[:], 1.0)

            wqkv_sb = sb.tile([P, CS, 384], dt.bfloat16, tag="wqkv")
            nc.sync.dma_start(wqkv_sb[:], wqkv_t.ap().rearrange("(o p) m -> p o m", p=P))

            xt_sb = sb.tile([P, NSLOT, CS, 512], dt.bfloat16, tag="xring")
            xt_r = xt_bf.ap().rearrange("(o p) t -> p o t", p=P)

            def xt_prefetch(c):
                tsl = slice(512 * c, 512 * (c + 1))
                nc.sync.dma_start(xt_sb[:, c % NSLOT, :, :], xt_r[:, :, tsl])

            xt_prefetch(0)   # chunks 1,2 after maskb; chunk 3 JIT at it1

            qa = [sb.tile([P, 512], dt.bfloat16, tag=f"qa{j}", name=f"qa{j}")
                  for j in range(2)]
            ka = [sb.tile([P, T], dt.bfloat16, tag=f"ka{j}", name=f"ka{j}")
                  for j in range(2)]
            v_sb = [sb.tile([P, 16, 65], dt.bfloat16, tag=f"v{j}", name=f"v{j}")
                    for j in range(2)]
            for j in range(2):
                nc.vector.memset(qa[j][64:128, :], 0.0)
                nc.vector.memset(ka[j][64:128, :], 0.0)
                nc.sync.dma_start(ka[j][64:68, :], aug_k.ap()[j])
                nc.vector.memset(v_sb[j][:, :, 64:65], 1.0)

            mb_sb = sb.tile([P, 4, 512], dt.bfloat16, tag="maskb")
            nc.sync.dma_start(mb_sb[:], maskb.ap().rearrange("r p f -> p r f"))
            for c in range(1, TT - 1):
                xt_prefetch(c)
            wout_sb = sb.tile([P, CS, P], dt.bfloat16, tag="wout")
            nc.sync.dma_start(wout_sb[:], wout_t.ap().rearrange("(o p) m -> p o m", p=P))
            wm_sb = sb.tile([P, CS, 1024], dt.bfloat16, tag="wm")
            nc.sync.dma_start(wm_sb[:], wm_t.ap().rearrange("(o p) m -> p o m", p=P))
            w3_sb = sb.tile([P, 4, 1024], dt.bfloat16, tag="w3")
            nc.sync.dma_start(w3_sb[:], w3_t.ap().rearrange("(o p) m -> p o m", p=P))

            # own-slice x2 in f32: written at oproj(d) (it=d+3), read at
            # resid(d) (it=d+5) -> 3-deep ring
            x2_sb = sb.tile([P, 3, 512], dt.float32, tag="x2f")

            # ---------- stage bodies ----------

            def qkv_chunk(c):
                sl4 = c % NSLOT
                tsl = slice(512 * c, 512 * (c + 1))
                xc = xt_sb[:, sl4, :, :]
                for j in range(2):
                    nc.sync.dma_start(qa[j][64:68, :], aug_q.ap()[j][:, tsl])
                ax = wk.tile([P, CS, 512], dt.bfloat16, tag="ax", bufs=1, name="ax")
                nc.scalar.activation(ax[:], xc, Act.Abs)
                csp = psF.tile([P, 512], dt.float32, tag="f", name="csp1")
                for o in range(CS):
                    nc.tensor.matmul(csp[:], ones128[:], ax[:, o, :],
                                     start=(o == 0), stop=(o == CS - 1))
                t1 = wk.tile([P, 512], dt.float32, tag="t1", bufs=1, name="t1")
                nc.vector.tensor_scalar(t1[:], csp[:], 1.0 / C, EPS,
                                        Alu.mult, Alu.add)
                rh = wk.tile([P, 512], dt.float32, tag="rh", bufs=1, name="rh")
                nc.vector.reciprocal_approx_fast(rh[:], t1[:])
                inv1 = wk.tile([P, 512], dt.bfloat16, tag="inv1", bufs=1,
                               name="inv1")
                nc.scalar.copy(inv1[:], rh[:])

                for ch in range(3):
                    pq = psS.tile([P, 512], dt.float32, tag="sc", name="pq")
                    for o in range(CS):
                        nc.tensor.matmul(pq[:], wqkv_sb[:, o, 128 * ch:128 * (ch + 1)],
                                         xc[:, o, :],
                                         start=(o == 0), stop=(o == CS - 1))
                    if ch == 0:
                        for j in range(2):
                            nc.vector.scalar_tensor_tensor(
                                qa[j][0:64, :], pq[64 * j:64 * j + 64, :], 1.0,
                                inv1[0:64, :], Alu.mult, Alu.mult)
                    elif ch == 1:
                        for j in range(2):
                            nc.vector.scalar_tensor_tensor(
                                ka[j][0:64, tsl], pq[64 * j:64 * j + 64, :], 1.0,
                                inv1[0:64, :], Alu.mult, Alu.mult)
                    else:
                        vt_w = wk.tile([P, 512], dt.bfloat16, tag="vt", bufs=1,
                                       name="vt_w")
                        nc.vector.scalar_tensor_tensor(
                            vt_w[:], pq[:], 1.0, inv1[:], Alu.mult, Alu.mult)
                        for u in range(4):
                            st = 4 * c + u
                            tp = psF.tile([P, P], dt.bfloat16, tag="f", name="tp")
                            nc.tensor.transpose(tp[:], vt_w[:, P * u:P * (u + 1)],
                                                ident[:])
                            for j in range(2):
                                nc.scalar.copy(v_sb[j][:, st, 0:64],
                                               tp[:, 64 * j:64 * j + 64])

            def scores_chunk(c):
                """Score matmuls + softmax numerator chains for BOTH heads
                of chunk c, at [P,2048] (= 4 k-blocks) chain granularity.
                The p4 tiles are consumed by av_chunk(c) in the NEXT
                iteration, so the vector chain has a whole iteration of
                slack and the tensor queue never waits on it.  The score
                PSUM is freed by the scalar copy to ss."""
                p4s = {0: [], 1: []}
                npair = 2 * c + 2
                for j in range(2):
                    for pp in range(npair // 2):      # pair of kp = 4 kblocks
                        ss = wk.tile([P, 2048], dt.bfloat16, tag="ss", bufs=3,
                                     name="ss")
                        for half4 in range(4):
                            k = 4 * pp + half4
                            sp = psS.tile([P, 512], dt.float32, tag="sc",
                                          name="sp")
                            diag = k >= 4 * c
                            nc.tensor.matmul(sp[:], ka[j][:, P * k:P * (k + 1)],
                                             qa[j][:, :], start=True,
                                             stop=not diag)
                            if diag:
                                rr = k - 4 * c
                                nc.tensor.matmul(sp[:], ident[:],
                                                 mb_sb[:, rr, :],
                                                 start=False, stop=True)
                            nc.scalar.copy(ss[:, 512 * half4:512 * (half4 + 1)],
                                           sp[:])
                        ab = wk.tile([P, 2048], dt.bfloat16, tag="ab", bufs=1,
                                     name="ab")
                        nc.scalar.activation(ab[:], ss[:], Act.Abs)
                        b = wk.tile([P, 2048], dt.float32, tag="b", bufs=1,
                                    name="b")
                        nc.vector.tensor_scalar(b[:], ab[:], 1.0, None, Alu.add)
                        r = wk.tile([P, 2048], dt.float32, tag="r", bufs=1,
                                    name="r")
                        nc.vector.reciprocal_approx_fast(r[:], b[:])
                        p4 = wk.tile([P, 2048], dt.bfloat16, tag="p4", bufs=8,
                                     name="p4")
                        nc.vector._custom_dve(P4_OP, out=p4[:], in0=ss[:],
                                              in1=r[:])
                        p4s[j].append(p4)
                return p4s

            def av_chunk(cm, p4s, attn_c):
                """AV matmuls for chunk cm (p4s produced last iteration) +
                denominators + attn_c assembly + AllGather."""
                nst = 4 * cm + 4
                for j in range(2):
                    pv = psV.tile([65, 512], dt.float32, tag="pv", name="pv")
                    for pp, p4 in enumerate(p4s[j]):
                        for half4 in range(4):
                            k = 4 * pp + half4
                            nc.tensor.matmul(
                                pv[:], v_sb[j][:, k, :],
                                p4[:, 512 * half4:512 * (half4 + 1)],
                                start=(k == 0), stop=(k == nst - 1))
                    de = wk.tile([1, 512], dt.float32, tag="de", name="de")
                    nc.vector.tensor_scalar(de[:], pv[64:65, :], 16.0 * EPS,
                                            None, Alu.add)
                    rd = wk.tile([1, 512], dt.float32, tag="rd", name="rd")
                    nc.vector.reciprocal_approx_fast(rd[:], de[:])
                    rdb = wk.tile([1, 512], dt.bfloat16, tag="rdb", name="rdb")
                    nc.scalar.copy(rdb[:], rd[:])
                    # broadcast rdb down 64 partitions via a rank-1 matmul
                    # (gpsimd stays free for the collective queue)
                    rbp = psF.tile([64, 512], dt.float32, tag="f", name="rbp")
                    nc.tensor.matmul(rbp[:], ones164[:], rdb[:],
                                     start=True, stop=True)
                    rdbb = wk.tile([64, 512], dt.bfloat16, tag="rdbb",
                                   name="rdbb")
                    nc.scalar.copy(rdbb[:], rbp[:])
                    nc.vector.tensor_tensor(attn_c[64 * j:64 * j + 64, :],
                                            pv[0:64, :], rdbb[:], Alu.mult)
                nc.sync.dma_start(attn_in.ap()[cm], attn_c[:])
                nc.gpsimd.collective_compute(
                    "AllGather", Alu.bypass, replica_groups=RG,
                    ins=[attn_in.ap()[cm]], outs=[attn_out.ap()[cm]])

            def oproj_chunk(d):
                """Own-slice out-proj + residual; bf16 copy AllGathered so
                every core gets the full x2."""
                tsl = slice(512 * d, 512 * (d + 1))
                af = wk.tile([P, CS, 512], dt.bfloat16, tag="af", bufs=1,
                             name="af")
                nc.sync.dma_start(af[:],
                                  attn_out.ap()[d].rearrange("(o p) t -> p o t", p=P))
                xsl = wk.tile([P, 512], dt.float32, tag="xsl", bufs=1,
                              name="xsl")
                nc.sync.dma_start(xsl[:], xt_sh.ap()[:, tsl])

                yo = psF.tile([P, 512], dt.float32, tag="f", name="yo")
                for o in range(CS):
                    nc.tensor.matmul(yo[:], wout_sb[:, o, :], af[:, o, :],
                                     start=(o == 0), stop=(o == CS - 1))
                nc.vector.tensor_tensor(x2_sb[:, d % 3, :], yo[:], xsl[:],
                                        Alu.add)
                x2b = wk.tile([P, 512], dt.bfloat16, tag="x2b", bufs=1,
                              name="x2b")
                nc.scalar.copy(x2b[:], x2_sb[:, d % 3, :])
                nc.sync.dma_start(h2_in.ap()[d], x2b[:])
                nc.gpsimd.collective_compute(
                    "AllGather", Alu.bypass, replica_groups=RG,
                    ins=[h2_in.ap()[d]], outs=[h2_out.ap()[d]])

            def ffn1_chunk(e, hid):
                """Gather the full x2 chunk, then FFN norm scalars +
                gate/val GEMMs + rational SwiGLU."""
                sl4 = e % NSLOT
                x2c = xt_sb[:, sl4, :, :]
                nc.sync.dma_start(
                    x2c, h2_out.ap()[e].rearrange("(o p) t -> p o t", p=P))
                ax2 = wk.tile([P, CS, 512], dt.bfloat16, tag="ax", bufs=1,
                              name="ax2")
                nc.scalar.activation(ax2[:], x2c, Act.Abs)
                csp2 = psF.tile([P, 512], dt.float32, tag="f", name="csp2")
                for o in range(CS):
                    nc.tensor.matmul(csp2[:], ones128[:], ax2[:, o, :],
                                     start=(o == 0), stop=(o == CS - 1))
                d2 = wk.tile([P, 512], dt.float32, tag="d2", bufs=1, name="d2")
                nc.vector.tensor_scalar(d2[:], csp2[:], 1.0 / C, EPS,
                                        Alu.mult, Alu.add)
                inv2 = wk.tile([P, 512], dt.float32, tag="inv2", bufs=1,
                               name="inv2")
                nc.vector.reciprocal_approx_fast(inv2[:], d2[:])
                isq = wk.tile([P, 512], dt.bfloat16, tag="isq", bufs=1,
                              name="isq")
                nc.scalar.square(isq[:], inv2[:])

                for fc in range(4):
                    gp = psF.tile([P, 512], dt.float32, tag="f", name="gp")
                    for o in range(CS):
                        nc.tensor.matmul(gp[:], wm_sb[:, o, 128 * fc:128 * (fc + 1)],
                                         x2c[:, o, :],
                                         start=(o == 0), stop=(o == CS - 1))
                    vp = psF.tile([P, 512], dt.float32, tag="f", name="vp")
                    for o in range(CS):
                        nc.tensor.matmul(vp[:],
                                         wm_sb[:, o, 512 + 128 * fc:512 + 128 * (fc + 1)],
                                         x2c[:, o, :],
                                         start=(o == 0), stop=(o == CS - 1))
                    ag = wk.tile([P, 512], dt.bfloat16, tag="ag", bufs=1, name="ag")
                    nc.scalar.activation(ag[:], gp[:], Act.Abs)
                    b2 = wk.tile([P, 512], dt.float32, tag="b2", bufs=1, name="b2")
                    nc.vector.tensor_tensor(b2[:], ag[:], d2[:], Alu.add)
                    r2 = wk.tile([P, 512], dt.float32, tag="r2", bufs=1,
                                 name="r2")
                    nc.vector.reciprocal_approx_fast(r2[:], b2[:])
                    gs = wk.tile([P, 512], dt.bfloat16, tag="gs", bufs=1,
                                 name="gs")
                    nc.vector._custom_dve(GSR_OP, out=gs[:], in0=gp[:], in1=r2[:])
                    vb = wk.tile([P, 512], dt.bfloat16, tag="vb", bufs=1, name="vb")
                    nc.scalar.copy(vb[:], vp[:])
                    vbs = wk.tile([P, 512], dt.bfloat16, tag="vbs", name="vbs")
                    nc.vector.tensor_tensor(vbs[:], vb[:], isq[:], Alu.mult)
                    nc.vector.tensor_tensor(hid[fc][:], gs[:], vbs[:], Alu.mult)

            def ffn2_chunk(e, hid):
                for jc in range(CS):
                    zp = psF.tile([P, 512], dt.float32, tag="f", name="zp")
                    for o in range(4):
                        nc.tensor.matmul(zp[:], w3_sb[:, o, 128 * jc:128 * (jc + 1)],
                                         hid[o][:],
                                         start=(o == 0), stop=(o == 3))
                    zs = wk.tile([P, 512], dt.bfloat16, tag="zs", bufs=1,
                                 name="zs")
                    nc.scalar.copy(zs[:], zp[:])
                    nc.sync.dma_start(rs_in[e, P * jc:P * (jc + 1), :], zs[:])

            def rs_collective(e):
                nc.gpsimd.collective_compute(
                    "ReduceScatter", Alu.add, replica_groups=RG,
                    ins=[rs_in.ap()[e]], outs=[rs_out.ap()[e]])

            def resid_chunk(c):
                tsl = slice(512 * c, 512 * (c + 1))
                rso = wk.tile([P, 512], dt.bfloat16, tag="rso", bufs=1,
                              name="rso")
                nc.sync.dma_start(rso[:], rs_out.ap()[c])
                of = wk.tile([P, 512], dt.float32, tag="of", bufs=1, name="of")
                nc.vector.tensor_tensor(of[:], rso[:], x2_sb[:, c % 3, :],
                                        Alu.add)
                nc.sync.dma_start(out[:, tsl], of[:])

            # ---------- the pipeline (depth 6) ----------
            # stage offsets: S1 qkv+scores(it), S2 AV+AGa(it-1),
            # S3 oproj+AGh(it-2), S4 ffn+RS(it-3), S5 resid(it-4).
            # The p4/attn_c tiles cross one iteration boundary, giving the
            # vector softmax chain a full iteration of slack.
            pend = {}
            for it in range(TT):
                c, cm, d, e = it, it - 1, it - 2, it - 3
                with nc.named_scope(f"it{it}"):
                    qkv_chunk(c)
                    if it == 1:
                        xt_prefetch(TT - 1)
                    if 0 <= cm:
                        p4s, attn_c = pend.pop(cm)
                        av_chunk(cm, p4s, attn_c)
                    if 0 <= d:
                        oproj_chunk(d)
                    hid = [wk.tile([P, 512], dt.bfloat16, tag=f"hid{fc}",
                                   bufs=1, name=f"hid{fc}") for fc in range(4)]
                    if 0 <= e:
                        ffn1_chunk(e, hid)
                        ffn2_chunk(e, hid)
                    p4s = scores_chunk(c)
                    attn_c = wk.tile([P, 512], dt.bfloat16, tag="attnc",
                                     name="attn_c")
                    pend[c] = (p4s, attn_c)
                    if 0 <= e:
                        rs_collective(e)

            # ---- drain: oproj(3) pulled into it4 so AGh(3) precedes RS(1)
            # on the collective queue and both tail FFNs fit in it5 ----
            with nc.named_scope("it4"):
                p4s, attn_c = pend.pop(3)
                av_chunk(3, p4s, attn_c)
                oproj_chunk(2)
                hid = [wk.tile([P, 512], dt.bfloat16, tag=f"hid{fc}",
                               bufs=1, name=f"hid{fc}") for fc in range(4)]
                ffn1_chunk(1, hid)
                ffn2_chunk(1, hid)
                resid_chunk(0)      # must precede oproj(3): shared x2 slot
                oproj_chunk(3)
                rs_collective(1)
            with nc.named_scope("it5"):
                hid = [wk.tile([P, 512], dt.bfloat16, tag=f"hid{fc}",
                               bufs=1, name=f"hid{fc}") for fc in range(4)]
                ffn1_chunk(2, hid)
                ffn2_chunk(2, hid)
                rs_collective(2)
                resid_chunk(1)
                hid = [wk.tile([P, 512], dt.bfloat16, tag=f"hid{fc}",
                               bufs=1, name=f"hid{fc}") for fc in range(4)]
                ffn1_chunk(3, hid)
                ffn2_chunk(3, hid)
                rs_collective(3)
            with nc.named_scope("it6"):
                resid_chunk(2)
                resid_chunk(3)

    nc.compile()
    return nc


def _get_program():
    global _PROGRAM
    if _PROGRAM is None:
        _PROGRAM = _build_program()
    return _PROGRAM


def kernel(x, w_qkv, w_out, w_merged, w3, norm1_w, norm2_w):
    global LAST_RESULTS
    from concourse.bass_utils import run_bass_kernel_spmd

    nc = _get_program()
    in_maps = _prepare_in_maps(x, w_qkv, w_out, w_merged, w3, norm1_w, norm2_w)
    res = run_bass_kernel_spmd(nc, in_maps, core_ids=list(range(NCORES)),
                               trace=TRACE)
    LAST_RESULTS = res
    yT = np.concatenate([res.results[i]["out"] for i in range(NCORES)], axis=0)
    return np.ascontiguousarray(yT.T)[None].astype(np.float32)


# revision 25
# speedup vs baseline: 1.0034x; 1.0034x over previous
"""TRN2 Bass kernel for nn_AlgebraicBlock (dense transformer block):
MR-norm -> QKV -> ALiBi attention w/ rational softmax -> out-proj residual ->
MR-norm -> rational SwiGLU FFN -> residual.   x: [1, 2048, 1024] f32.

Tensor-parallel over 8 NeuronCores, software-pipelined at 512-column
(t-chunk) granularity, 2 collectives per chunk (v1 had 3):

  it:   qkv+scores(it) -> [oproj+FFN1(it-1) interleaved with AV(it)] ->
        AllGather(attn it) -> FFN2(it-1) -> ReduceScatter(it-1) ->
        residual(it-2) -> DMA out

Key structural points vs v1:
- The FULL x2 = x + attn@w_out.T is computed redundantly on every core
  (kills v1's x2 AllGather); the x-residual rides the out-proj PSUM
  accumulation as an identity-matmul term.
- Causal mask applied as a -10000 bias identity-matmul into the score
  PSUM (like the reference), not a 0/1 multiply pass.
- Attention AV matmuls are emitted in two waves AFTER oproj / FFN1 so the
  FIFO tensor queue never waits on the vector softmax chain.
- The warmup collective is the very first instruction group so the
  one-time CC-init barrier overlaps weight DMAs and chunk-0 compute.

Rational softmax: RECIPROCAL_APPROX_FAST(1+|s|) then ANT_P4(s, r) =
((s*r+1)^2)^2 = 16*rsig(s)^4.  FFN rsig uses ANT_GSR(g, r) = (g*r+1)*g on
the unnormalized gate with b2 = mean|x2|+eps + |g|; inv2^2/2 rides the val
path (0.5 folded into w3 host-side).  All GEMMs bf16, f32 PSUM accum,
transposed [feature, T] layout.
"""

import numpy as np
import ml_dtypes

T, C, H, D, F = 2048, 1024, 16, 64, 4096
NCORES = 8
EPS = 1e-6
P = 128
TT = T // 512          # 4 t-chunks of 512
CS = C // P            # 8 c-subtiles
BF = ml_dtypes.bfloat16

TRACE = False          # set True by test.py for neuron-profile timing
LAST_RESULTS = None    # BassKernelResults of the last run (for test.py)

_PROGRAM = None
_DVE_OPS = None


def _bf16(x):
    return np.asarray(x, dtype=BF)


def _alibi_slopes():
    start = 2.0 ** (-8.0 / H)
    return (start ** np.arange(1, H + 1)).astype(np.float64)


def _register_dve_ops():
    """Register the two fused softmax/ffn DVE ops (idempotent)."""
    global _DVE_OPS
    if _DVE_OPS is not None:
        return _DVE_OPS
    import concourse.dve_ops as dops
    from concourse.dve_spec import Spec, Src0, Src1, One, lower, sq
    from concourse.dve_uop import DveOpSpec

    def _p4ref(in0, in1, c0, c1, c2):
        t = in0.astype(np.float32) * in1.astype(np.float32)
        return (((t + 1.0) ** 2) ** 2).astype(np.float32)

    def _gsrref(in0, in1, c0, c1, c2):
        t = in0.astype(np.float32) * in1.astype(np.float32)
        return ((t + 1.0) * in0.astype(np.float32)).astype(np.float32)

    defs = [
        ("ANT_P4", Spec(body=sq(sq(Src0 * Src1 + One)), reference=_p4ref)),
        ("ANT_GSR", Spec(body=(Src0 * Src1 + One) * Src0, reference=_gsrref)),
    ]
    made = []
    for name, spec in defs:
        if name in dops._SUB_OPCODE_FOR_NAME:
            made.append(next(o for o in dops.OPS if o.name == name))
            continue
        row = max(dops._SUB_OPCODE_FOR_NAME.values()) + 1
        assert row < 0x20
        dops._SUB_OPCODE_FOR_NAME[name] = row
        shas = {}
        for ver in ("v3", "v4"):
            u = lower(spec, ver=ver)
            shas[ver] = DveOpSpec(name=name, opcode=row, uops=u,
                                  rd1_en=True).sha(ver)
        op = dops.DveOp(name, spec, subdim=False, uops_sha=shas)
        dops.OPS.append(op)
        dops.CUSTOM_DVE_SPECS[name] = spec
        made.append(op)
    _DVE_OPS = tuple(made)
    return _DVE_OPS


def _prepare_in_maps(x, w_qkv, w_out, w_merged, w3, norm1_w, norm2_w):
    """Host-side sharding + weight preprocessing (layout/precision only)."""
    x = np.asarray(x, np.float32)[0]            # [T, C]
    xT = np.ascontiguousarray(x.T)              # [C, T]
    xt_bf = _bf16(xT)
    slopes = _alibi_slopes()

    pos = np.arange(T, dtype=np.float64)
    t_hi, t_lo = pos // 64, pos % 64

    Wn = np.asarray(w_qkv, np.float32) * np.asarray(norm1_w, np.float32)[None, :]
    w_out = np.asarray(w_out, np.float32)
    wm_n = np.asarray(w_merged, np.float32) * np.asarray(norm2_w, np.float32)[None, :]
    w3 = np.asarray(w3, np.float32) * 0.5       # rsig half-factor folded here

    in_maps = []
    for i in range(NCORES):
        h0, h1 = 2 * i, 2 * i + 1
        rows = []
        for h in (h0, h1):
            rows.append(Wn[64 * h:64 * h + 64, :] * 0.125)          # q (scaled)
        for h in (h0, h1):
            rows.append(Wn[C + 64 * h:C + 64 * h + 64, :])          # k
        for h in (h0, h1):
            rows.append(Wn[2 * C + 64 * h:2 * C + 64 * h + 64, :])  # v
        wqkv_t = _bf16(np.ascontiguousarray(np.concatenate(rows, 0).T))  # [1024, 384]

        wout_t = _bf16(np.ascontiguousarray(w_out[P * i:P * (i + 1), :].T))  # [1024, 128]

        gsl = slice(512 * i, 512 * (i + 1))
        wm = np.concatenate([wm_n[gsl, :], wm_n[F:][gsl, :]], 0)     # [1024, C]
        wm_t = _bf16(np.ascontiguousarray(wm.T))                     # [1024, 1024]
        w3_t = _bf16(np.ascontiguousarray(w3[:, gsl].T))             # [512, 1024]

        aug_q = np.zeros((2, 4, T), np.float64)
        aug_k = np.zeros((2, 4, T), np.float64)
        for j, h in enumerate((h0, h1)):
            sl = float(_bf16(slopes[h]))
            aug_q[j, 0] = -t_hi
            aug_q[j, 1] = -t_lo
            aug_q[j, 2] = sl * 64
            aug_q[j, 3] = sl
            aug_k[j, 0] = sl * 64
            aug_k[j, 1] = sl
            aug_k[j, 2] = t_hi
            aug_k[j, 3] = t_lo

        # causal-mask BIAS tiles: -10000 where key > query, else 0.
        # maskb[rr][ss, tq]: key = 128*(4c+rr)+ss vs query = 512c+tq
        # -> masked iff 128*rr + ss > tq (c cancels).
        maskb = np.zeros((4, P, 512), np.float64)
        for rr in range(4):
            s_idx = P * rr + np.arange(P)[:, None]
            maskb[rr] = np.where(s_idx > np.arange(512)[None, :], -10000.0, 0.0)

        in_maps.append({
            "xt_bf": xt_bf,
            "xt_sh": np.ascontiguousarray(xT[P * i:P * (i + 1)]),
            "wqkv_t": wqkv_t,
            "wout_t": wout_t,
            "wm_t": wm_t,
            "w3_t": w3_t,
            "aug_q": _bf16(aug_q),
            "aug_k": _bf16(aug_k),
            "maskb": _bf16(maskb),
        })
    return in_maps


def _build_program():
    import concourse.bass as bass
    import concourse.mybir as mybir
    import concourse.tile as tile
    from concourse import bacc
    from concourse.masks import make_identity

    P4_OP, GSR_OP = _register_dve_ops()

    dt = mybir.dt
    Alu = mybir.AluOpType
    Act = mybir.ActivationFunctionType

    nc = bacc.Bacc("TRN2", target_bir_lowering=False, debug=False,
                   enable_asserts=True, num_devices=NCORES)

    # I/O
    xt_bf = nc.dram_tensor("xt_bf", [C, T], dt.bfloat16, kind="ExternalInput")
    xt_sh = nc.dram_tensor("xt_sh", [P, T], dt.float32, kind="ExternalInput")
    wqkv_t = nc.dram_tensor("wqkv_t", [C, 384], dt.bfloat16, kind="ExternalInput")
    wout_t = nc.dram_tensor("wout_t", [C, P], dt.bfloat16, kind="ExternalInput")
    wm_t = nc.dram_tensor("wm_t", [C, 1024], dt.bfloat16, kind="ExternalInput")
    w3_t = nc.dram_tensor("w3_t", [512, 1024], dt.bfloat16, kind="ExternalInput")
    aug_q = nc.dram_tensor("aug_q", [2, 4, T], dt.bfloat16, kind="ExternalInput")
    aug_k = nc.dram_tensor("aug_k", [2, 4, T], dt.bfloat16, kind="ExternalInput")
    maskb = nc.dram_tensor("maskb", [4, P, 512], dt.bfloat16, kind="ExternalInput")
    out = nc.dram_tensor("out", [P, T], dt.float32, kind="ExternalOutput")

    # internal DRAM (collective bounces)
    warm_in = nc.dram_tensor("warm_in", [1, 64], dt.float32)
    warm_out = nc.dram_tensor("warm_out", [1, 512], dt.float32, addr_space="Shared")
    attn_in = nc.dram_tensor("attn_in", [TT, P, 512], dt.bfloat16)
    attn_out = nc.dram_tensor("attn_out", [TT, C, 512], dt.bfloat16, addr_space="Shared")
    h2_in = nc.dram_tensor("h2_in", [TT, P, 512], dt.bfloat16)
    h2_out = nc.dram_tensor("h2_out", [TT, C, 512], dt.bfloat16, addr_space="Shared")
    rs_in = nc.dram_tensor("rs_in", [TT, C, 512], dt.bfloat16)
    rs_out = nc.dram_tensor("rs_out", [TT, P, 512], dt.bfloat16)

    RG = [list(range(NCORES))]
    NSLOT = 3  # x / gathered-x2 chunk ring

    with tile.TileContext(nc, num_cores=NCORES) as tc:
        with (
            tc.tile_pool(name="sb", bufs=1) as sb,
            tc.tile_pool(name="wk", bufs=2) as wk,
            tc.tile_pool(name="psS", bufs=3, space="PSUM") as psS,
            tc.tile_pool(name="psV", bufs=2, space="PSUM") as psV,
            tc.tile_pool(name="psF", bufs=3, space="PSUM") as psF,
        ):
            # ---------- warmup collective FIRST ----------
            # No input dependency: gather uninitialized scratch.  The
            # trigger fires at ~0 on every core (in some runs the tiny
            # warm_in DMA sat ~70us behind queue init, delaying the CC-init
            # rendezvous and the whole collective pipeline).
            nc.gpsimd.collective_compute(
                "AllGather", Alu.bypass, replica_groups=RG,
                ins=[warm_in.ap()], outs=[warm_out.ap()])

            # ---------- constants + weight DMA ----------
            # DMA order matters for the first-iteration critical path:
            # qkv weights + x chunk 0 first.
            ident = sb.tile([P, P], dt.bfloat16, tag="ident")
            make_identity(nc, ident[:])
            ones128 = sb.tile([P, P], dt.bfloat16, tag="ones")
            nc.vector.memset(ones128[:], 1.0)
            ones164 = sb.tile([1, 64], dt.bfloat16, tag="ones164")
            nc.vector.memset(ones164[:], 1.0)

            wqkv_sb = sb.tile([P, CS, 384], dt.bfloat16, tag="wqkv")
            nc.sync.dma_start(wqkv_sb[:], wqkv_t.ap().rearrange("(o p) m -> p o m", p=P))

            xt_sb = sb.tile([P, NSLOT, CS, 512], dt.bfloat16, tag="xring")
            xt_r = xt_bf.ap().rearrange("(o p) t -> p o t", p=P)

            def xt_prefetch(c):
                tsl = slice(512 * c, 512 * (c + 1))
                nc.sync.dma_start(xt_sb[:, c % NSLOT, :, :], xt_r[:, :, tsl])

            xt_prefetch(0)   # chunks 1,2 after maskb; chunk 3 JIT at it1

            qa = [sb.tile([P, 512], dt.bfloat16, tag=f"qa{j}", name=f"qa{j}")
                  for j in range(2)]
            ka = [sb.tile([P, T], dt.bfloat16, tag=f"ka{j}", name=f"ka{j}")
                  for j in range(2)]
            v_sb = [sb.tile([P, 16, 65], dt.bfloat16, tag=f"v{j}", name=f"v{j}")
                    for j in range(2)]
            for j in range(2):
                nc.vector.memset(qa[j][64:128, :], 0.0)
                nc.vector.memset(ka[j][64:128, :], 0.0)
                nc.sync.dma_start(ka[j][64:68, :], aug_k.ap()[j])
                nc.vector.memset(v_sb[j][:, :, 64:65], 1.0)

            mb_sb = sb.tile([P, 4, 512], dt.bfloat16, tag="maskb")
            nc.sync.dma_start(mb_sb[:], maskb.ap().rearrange("r p f -> p r f"))
            for c in range(1, TT - 1):
                xt_prefetch(c)
            wout_sb = sb.tile([P, CS, P], dt.bfloat16, tag="wout")
            nc.sync.dma_start(wout_sb[:], wout_t.ap().rearrange("(o p) m -> p o m", p=P))
            wm_sb = sb.tile([P, CS, 1024], dt.bfloat16, tag="wm")
            nc.sync.dma_start(wm_sb[:], wm_t.ap().rearrange("(o p) m -> p o m", p=P))
            w3_sb = sb.tile([P, 4, 1024], dt.bfloat16, tag="w3")
            nc.sync.dma_start(w3_sb[:], w3_t.ap().rearrange("(o p) m -> p o m", p=P))

            # own-slice x2 in f32: written at oproj(d) (it=d+3), read at
            # resid(d) (it=d+5) -> 3-deep ring
            x2_sb = sb.tile([P, 3, 512], dt.float32, tag="x2f")

            # ---------- stage bodies ----------

            def qkv_chunk(c):
                sl4 = c % NSLOT
                tsl = slice(512 * c, 512 * (c + 1))
                xc = xt_sb[:, sl4, :, :]
                for j in range(2):
                    nc.sync.dma_start(qa[j][64:68, :], aug_q.ap()[j][:, tsl])
                ax = wk.tile([P, CS, 512], dt.bfloat16, tag="ax", bufs=1, name="ax")
                nc.scalar.activation(ax[:], xc, Act.Abs)
                csp = psF.tile([P, 512], dt.float32, tag="f", name="csp1")
                for o in range(CS):
                    nc.tensor.matmul(csp[:], ones128[:], ax[:, o, :],
                                     start=(o == 0), stop=(o == CS - 1))
                t1 = wk.tile([P, 512], dt.float32, tag="t1", bufs=1, name="t1")
                nc.vector.tensor_scalar(t1[:], csp[:], 1.0 / C, EPS,
                                        Alu.mult, Alu.add)
                rh = wk.tile([P, 512], dt.float32, tag="rh", bufs=1, name="rh")
                nc.vector.reciprocal_approx_fast(rh[:], t1[:])
                inv1 = wk.tile([P, 512], dt.bfloat16, tag="inv1", bufs=1,
                               name="inv1")
                nc.scalar.copy(inv1[:], rh[:])

                for ch in range(3):
                    pq = psS.tile([P, 512], dt.float32, tag="sc", name="pq")
                    for o in range(CS):
                        nc.tensor.matmul(pq[:], wqkv_sb[:, o, 128 * ch:128 * (ch + 1)],
                                         xc[:, o, :],
                                         start=(o == 0), stop=(o == CS - 1))
                    if ch == 0:
                        for j in range(2):
                            nc.vector.scalar_tensor_tensor(
                                qa[j][0:64, :], pq[64 * j:64 * j + 64, :], 1.0,
                                inv1[0:64, :], Alu.mult, Alu.mult)
                    elif ch == 1:
                        for j in range(2):
                            nc.vector.scalar_tensor_tensor(
                                ka[j][0:64, tsl], pq[64 * j:64 * j + 64, :], 1.0,
                                inv1[0:64, :], Alu.mult, Alu.mult)
                    else:
                        vt_w = wk.tile([P, 512], dt.bfloat16, tag="vt", bufs=1,
                                       name="vt_w")
                        nc.vector.scalar_tensor_tensor(
                            vt_w[:], pq[:], 1.0, inv1[:], Alu.mult, Alu.mult)
                        for u in range(4):
                            st = 4 * c + u
                            tp = psF.tile([P, P], dt.bfloat16, tag="f", name="tp")
                            nc.tensor.transpose(tp[:], vt_w[:, P * u:P * (u + 1)],
                                                ident[:])
                            for j in range(2):
                                nc.scalar.copy(v_sb[j][:, st, 0:64],
                                               tp[:, 64 * j:64 * j + 64])

            def scores_chunk(c):
                """Score matmuls + softmax numerator chains for BOTH heads
                of chunk c, at [P,2048] (= 4 k-blocks) chain granularity.
                The p4 tiles are consumed by av_chunk(c) in the NEXT
                iteration, so the vector chain has a whole iteration of
                slack and the tensor queue never waits on it.  The score
                PSUM is freed by the scalar copy to ss."""
                p4s = {0: [], 1: []}
                npair = 2 * c + 2
                for j in range(2):
                    for pp in range(npair // 2):      # pair of kp = 4 kblocks
                        ss = wk.tile([P, 2048], dt.bfloat16, tag="ss", bufs=3,
                                     name="ss")
                        for half4 in range(4):
                            k = 4 * pp + half4
                            sp = psS.tile([P, 512], dt.float32, tag="sc",
                                          name="sp")
                            diag = k >= 4 * c
                            nc.tensor.matmul(sp[:], ka[j][:, P * k:P * (k + 1)],
                                             qa[j][:, :], start=True,
                                             stop=not diag)
                            if diag:
                                rr = k - 4 * c
                                nc.tensor.matmul(sp[:], ident[:],
                                                 mb_sb[:, rr, :],
                                                 start=False, stop=True)
                            nc.scalar.copy(ss[:, 512 * half4:512 * (half4 + 1)],
                                           sp[:])
                        ab = wk.tile([P, 2048], dt.bfloat16, tag="ab", bufs=1,
                                     name="ab")
                        nc.scalar.activation(ab[:], ss[:], Act.Abs)
                        b = wk.tile([P, 2048], dt.float32, tag="b", bufs=1,
                                    name="b")
                        nc.vector.tensor_scalar(b[:], ab[:], 1.0, None, Alu.add)
                        r = wk.tile([P, 2048], dt.float32, tag="r", bufs=1,
                                    name="r")
                        nc.vector.reciprocal_approx_fast(r[:], b[:])
                        p4 = wk.tile([P, 2048], dt.bfloat16, tag="p4", bufs=8,
                                     name="p4")
                        nc.vector._custom_dve(P4_OP, out=p4[:], in0=ss[:],
                                              in1=r[:])
                        p4s[j].append(p4)
                return p4s

            def av_chunk(cm, p4s, attn_c):
                """AV matmuls for chunk cm (p4s produced last iteration) +
                denominators + attn_c assembly + AllGather."""
                nst = 4 * cm + 4
                for j in range(2):
                    pv = psV.tile([65, 512], dt.float32, tag="pv", name="pv")
                    for pp, p4 in enumerate(p4s[j]):
                        for half4 in range(4):
                            k = 4 * pp + half4
                            nc.tensor.matmul(
                                pv[:], v_sb[j][:, k, :],
                                p4[:, 512 * half4:512 * (half4 + 1)],
                                start=(k == 0), stop=(k == nst - 1))
                    de = wk.tile([1, 512], dt.float32, tag="de", name="de")
                    nc.vector.tensor_scalar(de[:], pv[64:65, :], 16.0 * EPS,
                                            None, Alu.add)
                    rd = wk.tile([1, 512], dt.float32, tag="rd", name="rd")
                    nc.vector.reciprocal_approx_fast(rd[:], de[:])
                    rdb = wk.tile([1, 512], dt.bfloat16, tag="rdb", name="rdb")
                    nc.scalar.copy(rdb[:], rd[:])
                    # broadcast rdb down 64 partitions via a rank-1 matmul
                    # (gpsimd stays free for the collective queue)
                    rbp = psF.tile([64, 512], dt.float32, tag="f", name="rbp")
                    nc.tensor.matmul(rbp[:], ones164[:], rdb[:],
                                     start=True, stop=True)
                    rdbb = wk.tile([64, 512], dt.bfloat16, tag="rdbb",
                                   name="rdbb")
                    nc.scalar.copy(rdbb[:], rbp[:])
                    nc.vector.tensor_tensor(attn_c[64 * j:64 * j + 64, :],
                                            pv[0:64, :], rdbb[:], Alu.mult)
                nc.sync.dma_start(attn_in.ap()[cm], attn_c[:])
                nc.gpsimd.collective_compute(
                    "AllGather", Alu.bypass, replica_groups=RG,
                    ins=[attn_in.ap()[cm]], outs=[attn_out.ap()[cm]])

            def oproj_chunk(d):
                """Own-slice out-proj + residual; bf16 copy AllGathered so
                every core gets the full x2."""
                tsl = slice(512 * d, 512 * (d + 1))
                af = wk.tile([P, CS, 512], dt.bfloat16, tag="af", bufs=1,
                             name="af")
                nc.sync.dma_start(af[:],
                                  attn_out.ap()[d].rearrange("(o p) t -> p o t", p=P))
                xsl = wk.tile([P, 512], dt.float32, tag="xsl", bufs=1,
                              name="xsl")
                nc.sync.dma_start(xsl[:], xt_sh.ap()[:, tsl])

                yo = psF.tile([P, 512], dt.float32, tag="f", name="yo")
                for o in range(CS):
                    nc.tensor.matmul(yo[:], wout_sb[:, o, :], af[:, o, :],
                                     start=(o == 0), stop=(o == CS - 1))
                nc.vector.tensor_tensor(x2_sb[:, d % 3, :], yo[:], xsl[:],
                                        Alu.add)
                x2b = wk.tile([P, 512], dt.bfloat16, tag="x2b", bufs=1,
                              name="x2b")
                nc.scalar.copy(x2b[:], x2_sb[:, d % 3, :])
                nc.sync.dma_start(h2_in.ap()[d], x2b[:])
                nc.gpsimd.collective_compute(
                    "AllGather", Alu.bypass, replica_groups=RG,
                    ins=[h2_in.ap()[d]], outs=[h2_out.ap()[d]])

            def ffn1_chunk(e, hid):
                """Gather the full x2 chunk, then FFN norm scalars +
                gate/val GEMMs + rational SwiGLU."""
                sl4 = e % NSLOT
                x2c = xt_sb[:, sl4, :, :]
                nc.sync.dma_start(
                    x2c, h2_out.ap()[e].rearrange("(o p) t -> p o t", p=P))
                ax2 = wk.tile([P, CS, 512], dt.bfloat16, tag="ax", bufs=1,
                              name="ax2")
                nc.scalar.activation(ax2[:], x2c, Act.Abs)
                csp2 = psF.tile([P, 512], dt.float32, tag="f", name="csp2")
                for o in range(CS):
                    nc.tensor.matmul(csp2[:], ones128[:], ax2[:, o, :],
                                     start=(o == 0), stop=(o == CS - 1))
                d2 = wk.tile([P, 512], dt.float32, tag="d2", bufs=1, name="d2")
                nc.vector.tensor_scalar(d2[:], csp2[:], 1.0 / C, EPS,
                                        Alu.mult, Alu.add)
                inv2 = wk.tile([P, 512], dt.float32, tag="inv2", bufs=1,
                               name="inv2")
                nc.vector.reciprocal_approx_fast(inv2[:], d2[:])
                isq = wk.tile([P, 512], dt.bfloat16, tag="isq", bufs=1,
                              name="isq")
                nc.scalar.square(isq[:], inv2[:])

                for fc in range(4):
                    gp = psF.tile([P, 512], dt.float32, tag="f", name="gp")
                    for o in range(CS):
                        nc.tensor.matmul(gp[:], wm_sb[:, o, 128 * fc:128 * (fc + 1)],
                                         x2c[:, o, :],
                                         start=(o == 0), stop=(o == CS - 1))
                    vp = psF.tile([P, 512], dt.float32, tag="f", name="vp")
                    for o in range(CS):
                        nc.tensor.matmul(vp[:],
                                         wm_sb[:, o, 512 + 128 * fc:512 + 128 * (fc + 1)],
                                         x2c[:, o, :],
                                         start=(o == 0), stop=(o == CS - 1))
                    ag = wk.tile([P, 512], dt.bfloat16, tag="ag", bufs=1, name="ag")
                    nc.scalar.activation(ag[:], gp[:], Act.Abs)
                    b2 = wk.tile([P, 512], dt.float32, tag="b2", bufs=1, name="b2")
                    nc.vector.tensor_tensor(b2[:], ag[:], d2[:], Alu.add)
                    r2 = wk.tile([P, 512], dt.float32, tag="r2", bufs=1,
                                 name="r2")
                    nc.vector.reciprocal_approx_fast(r2[:], b2[:])
                    gs = wk.tile([P, 512], dt.bfloat16, tag="gs", bufs=1,
                                 name="gs")
                    nc.vector._custom_dve(GSR_OP, out=gs[:], in0=gp[:], in1=r2[:])
                    vb = wk.tile([P, 512], dt.bfloat16, tag="vb", bufs=1, name="vb")
                    nc.scalar.copy(vb[:], vp[:])
                    vbs = wk.tile([P, 512], dt.bfloat16, tag="vbs", name="vbs")
                    nc.vector.tensor_tensor(vbs[:], vb[:], isq[:], Alu.mult)
                    nc.vector.tensor_tensor(hid[fc][:], gs[:], vbs[:], Alu.mult)

            def ffn2_chunk(e, hid):
                for jc in range(CS):
                    zp = psF.tile([P, 512], dt.float32, tag="f", name="zp")
                    for o in range(4):
                        nc.tensor.matmul(zp[:], w3_sb[:, o, 128 * jc:128 * (jc + 1)],
                                         hid[o][:],
                                         start=(o == 0), stop=(o == 3))
                    zs = wk.tile([P, 512], dt.bfloat16, tag="zs", bufs=1,
                                 name="zs")
                    nc.scalar.copy(zs[:], zp[:])
                    nc.sync.dma_start(rs_in[e, P * jc:P * (jc + 1), :], zs[:])

            def rs_collective(e):
                nc.gpsimd.collective_compute(
                    "ReduceScatter", Alu.add, replica_groups=RG,
                    ins=[rs_in.ap()[e]], outs=[rs_out.ap()[e]])

            def resid_chunk(c):
                tsl = slice(512 * c, 512 * (c + 1))
                rso = wk.tile([P, 512], dt.bfloat16, tag="rso", bufs=1,
                              name="rso")
                nc.sync.dma_start(rso[:], rs_out.ap()[c])
                of = wk.tile([P, 512], dt.float32, tag="of", bufs=1, name="of")
                nc.vector.tensor_tensor(of[:], rso[:], x2_sb[:, c % 3, :],
                                        Alu.add)
                nc.sync.dma_start(out[:, tsl], of[:])

            # ---------- the pipeline (depth 6) ----------
            # stage offsets: S1 qkv+scores(it), S2 AV+AGa(it-1),
            # S3 oproj+AGh(it-2), S4 ffn+RS(it-3), S5 resid(it-4).
            # The p4/attn_c tiles cross one iteration boundary, giving the
            # vector softmax chain a full iteration of slack.
            pend = {}
            for it in range(TT):
                c, cm, d, e = it, it - 1, it - 2, it - 3
                with nc.named_scope(f"it{it}"):
                    qkv_chunk(c)
                    if it == 1:
                        xt_prefetch(TT - 1)
                    if 0 <= cm:
                        p4s, attn_c = pend.pop(cm)
                        av_chunk(cm, p4s, attn_c)
                    if 0 <= d:
                        oproj_chunk(d)
                    hid = [wk.tile([P, 512], dt.bfloat16, tag=f"hid{fc}",
                                   bufs=1, name=f"hid{fc}") for fc in range(4)]
                    if 0 <= e:
                        ffn1_chunk(e, hid)
                        ffn2_chunk(e, hid)
                    p4s = scores_chunk(c)
                    attn_c = wk.tile([P, 512], dt.bfloat16, tag="attnc",
                                     name="attn_c")
                    pend[c] = (p4s, attn_c)
                    if 0 <= e:
                        rs_collective(e)

            # ---- drain: oproj(3) pulled into it4 so AGh(3) precedes RS(1)
            # on the collective queue and both tail FFNs fit in it5 ----
            with nc.named_scope("it4"):
                p4s, attn_c = pend.pop(3)
                av_chunk(3, p4s, attn_c)
                oproj_chunk(2)
                hid = [wk.tile([P, 512], dt.bfloat16, tag=f"hid{fc}",
                               bufs=1, name=f"hid{fc}") for fc in range(4)]
                ffn1_chunk(1, hid)
                ffn2_chunk(1, hid)
                resid_chunk(0)      # must precede oproj(3): shared x2 slot
                oproj_chunk(3)
                rs_collective(1)
            with nc.named_scope("it5"):
                hid = [wk.tile([P, 512], dt.bfloat16, tag=f"hid{fc}",
                               bufs=1, name=f"hid{fc}") for fc in range(4)]
                ffn1_chunk(2, hid)
                ffn2_chunk(2, hid)
                rs_collective(2)
                resid_chunk(1)
                hid = [wk.tile([P, 512], dt.bfloat16, tag=f"hid{fc}",
                               bufs=1, name=f"hid{fc}") for fc in range(4)]
                ffn1_chunk(3, hid)
                ffn2_chunk(3, hid)
                rs_collective(3)
            with nc.named_scope("it6"):
                resid_chunk(2)
                resid_chunk(3)

    nc.compile()
    return nc


def _get_program():
    global _PROGRAM
    if _PROGRAM is None:
        _PROGRAM = _build_program()
    return _PROGRAM


def kernel(x, w_qkv, w_out, w_merged, w3, norm1_w, norm2_w):
    global LAST_RESULTS
    from concourse.bass_utils import run_bass_kernel_spmd

    nc = _get_program()
    in_maps = _prepare_in_maps(x, w_qkv, w_out, w_merged, w3, norm1_w, norm2_w)
    res = run_bass_kernel_spmd(nc, in_maps, core_ids=list(range(NCORES)),
                               trace=TRACE)
    LAST_RESULTS = res
    yT = np.concatenate([res.results[i]["out"] for i in range(NCORES)], axis=0)
    return np.ascontiguousarray(yT.T)[None].astype(np.float32)
